# revision 21
# baseline (speedup 1.0000x reference)
"""Trainium2 Bass kernel for the DiffSSM block.

Data-parallel over batch B=8 across 8 NeuronCores (one batch element per
core). All heavy compute runs on the TensorEngine in bf16 with fp32 PSUM.

The bidirectional SSM global convolution is computed with a chunked
low-rank decomposition instead of a dense L x L Toeplitz matmul:
  - intra-chunk (Q=128): dense local Toeplitz, one 128x128 matmul per
    (chunk, dblock), evicted through ScalarE with the noise scale folded;
  - inter-chunk: per-chunk mode states S = V^T h (V packs 63 fwd modes +
    ones + 63 bwd modes + ones, one mode per direction folded into the
    D-term), an in-place fp16 prefix/suffix scan over chunks on the
    VectorE, and a 128x128 expand matmul per (chunk, dblock) accumulated
    into the mix.
Per-core device dataflow (L=2048, D=1024, P=128):
  A: per lt: h = x @ Wi (+bi), LN1 stats from PSUM, normalize straight to
     bf16 hln (g1/b1 folded into downstream consumers), SBUF->SBUF DMA
     transpose into hlnT, SSM state + intra-chunk matmuls.
  scan: fwd/bwd chunk scans (DVE, fp16, in-place).
  conv1: kernel-3 conv as 3 shifted matmuls, Silu eviction on ScalarE.
  expand: inter-chunk matmuls, DVE eviction adds into h2T with ns scale.
  conv2: accumulate into h2T (+bc2).
  F: y = h2 @ Wo (+bo), LN2 from PSUM, residual add, DMA out fp32.
"""

import math

import numpy as np
import ml_dtypes

_BF16 = ml_dtypes.bfloat16
_F16 = np.float16

_L, _D, _B = 2048, 1024, 8
_SV = 256.0

_cache = {}


def _build(L, D, n_cores, flags):
    import concourse.bacc as bacc
    import concourse.bass as bass
    import concourse.tile as tile
    from concourse import mybir

    use_bi, use_bo, use_g2, use_b2, use_b1mix = flags

    f32 = mybir.dt.float32
    bf16 = mybir.dt.bfloat16
    f16 = mybir.dt.float16
    AF = mybir.ActivationFunctionType
    OP = mybir.AluOpType

    P = 128
    KT = D // P            # feature tiles
    LT = L // P            # sequence chunks (Q = P)
    ND = 512               # matmul free chunk along features
    NF = 512               # conv free chunk along sequence
    EH = D // ND
    LC = L // NF
    XB = 8                 # xT column blocks
    XW = L // XB

    nc = bacc.Bacc("TRN2", target_bir_lowering=False, debug=False,
                   num_devices=n_cores)

    x_res = nc.dram_tensor("x_res", (L, D), f32, kind="ExternalInput").ap()
    xT = nc.dram_tensor("xT", (D, L), bf16, kind="ExternalInput").ap()
    Wi = nc.dram_tensor("Wi", (D, D), bf16, kind="ExternalInput").ap()
    w1T = nc.dram_tensor("w1T", (KT, P, 3, D), bf16, kind="ExternalInput").ap()
    w2T = nc.dram_tensor("w2T", (KT, P, 3, D), bf16, kind="ExternalInput").ap()
    Wo = nc.dram_tensor("Wo", (D, D), bf16, kind="ExternalInput").ap()
    Vp = nc.dram_tensor("Vp", (P, P), bf16, kind="ExternalInput").ap()
    Up = nc.dram_tensor("Up", (P, P), f16, kind="ExternalInput").ap()
    TlT = nc.dram_tensor("TlT", (P, P), bf16, kind="ExternalInput").ap()
    dkc = nc.dram_tensor("dkc", (P, 1), f32, kind="ExternalInput").ap()
    nsc = nc.dram_tensor("nsc", (P, KT), f32, kind="ExternalInput").ap()
    bc1c = nc.dram_tensor("bc1c", (P, KT), f32, kind="ExternalInput").ap()
    bc2c = nc.dram_tensor("bc2c", (P, KT), f32, kind="ExternalInput").ap()
    if use_bi:
        biR = nc.dram_tensor("biR", (1, D), f32, kind="ExternalInput").ap()
    if use_bo:
        boR = nc.dram_tensor("boR", (1, D), f32, kind="ExternalInput").ap()
    if use_g2:
        g2R = nc.dram_tensor("g2R", (1, D), f32, kind="ExternalInput").ap()
    if use_b2:
        b2R = nc.dram_tensor("b2R", (1, D), f32, kind="ExternalInput").ap()
    if use_b1mix:
        rsR = nc.dram_tensor("rsR", (1, L), f32, kind="ExternalInput").ap()
        nb1c = nc.dram_tensor("nb1c", (P, KT), f32,
                              kind="ExternalInput").ap()
    out = nc.dram_tensor("out", (L, D), f32, kind="ExternalOutput").ap()

    with tile.TileContext(nc) as tc:
        const = tc.alloc_tile_pool(name="const", bufs=1)
        psA = tc.alloc_tile_pool(name="psA", bufs=4, space="PSUM")
        psI = tc.alloc_tile_pool(name="psI", bufs=4, space="PSUM")
        statp = tc.alloc_tile_pool(name="stat", bufs=4)

        # ---- constants (scalar DMA queue; sync queue is busy with xT/Wi) ----
        Vp_sb = const.tile([P, P], bf16)
        nc.scalar.dma_start(out=Vp_sb[:], in_=Vp)
        U_sb = const.tile([P, P], f16)
        nc.scalar.dma_start(out=U_sb[:], in_=Up)
        Tl_sb = const.tile([P, P], bf16)
        nc.scalar.dma_start(out=Tl_sb[:], in_=TlT)
        dk_sb = const.tile([P, 1], f32)
        nc.scalar.dma_start(out=dk_sb[:], in_=dkc)
        ns_sb = const.tile([P, KT], f32)
        nc.scalar.dma_start(out=ns_sb[:], in_=nsc)
        bc1_sb = const.tile([P, KT], f32)
        nc.scalar.dma_start(out=bc1_sb[:], in_=bc1c)
        bc2_sb = const.tile([P, KT], f32)
        nc.scalar.dma_start(out=bc2_sb[:], in_=bc2c)
        eps_sb = const.tile([P, 1], f32)
        nc.vector.memset(eps_sb[:], 1e-5)
        ones_sb = const.tile([1, P], bf16)
        nc.vector.memset(ones_sb[:], 1.0)
        if use_bi:
            bi_bf = const.tile([1, D], bf16)
            nc.gpsimd.dma_start(out=bi_bf[:], in_=biR)
        if use_bo:
            bo_bf = const.tile([1, D], bf16)
            nc.gpsimd.dma_start(out=bo_bf[:], in_=boR)
        if use_g2:
            g2_rep = const.tile([P, D], f32)
            g2b = bass.AP(tensor=g2R.tensor, offset=g2R.offset,
                          ap=[[0, P]] + list(g2R.ap)[1:])
            nc.gpsimd.dma_start(out=g2_rep[:], in_=g2b)
        if use_b2:
            b2_rep = const.tile([P, D], f32)
            b2b = bass.AP(tensor=b2R.tensor, offset=b2R.offset,
                          ap=[[0, P]] + list(b2R.ap)[1:])
            nc.gpsimd.dma_start(out=b2_rep[:], in_=b2b)
        if use_b1mix:
            rs_rep = const.tile([P, L], f32)
            rsb = bass.AP(tensor=rsR.tensor, offset=rsR.offset,
                          ap=[[0, P]] + list(rsR.ap)[1:])
            nc.gpsimd.dma_start(out=rs_rep[:], in_=rsb)
            nb1_sb = const.tile([P, KT], f32)
            nc.gpsimd.dma_start(out=nb1_sb[:], in_=nb1c)

        # ---- persistent big buffers (left stack, LIFO) ----
        h2T_pool = tc.alloc_tile_pool(name="h2T", bufs=1)
        h2T_sb = h2T_pool.tile([P, KT, L], bf16)
        hlnT_pool = tc.alloc_tile_pool(name="hlnT", bufs=1)
        hlnT_sb = hlnT_pool.tile([P, KT, L], bf16)
        # w1 storage is reused for w2 after conv1; wi storage for Wo.
        w1p = tc.alloc_tile_pool(name="w1", bufs=1)
        w1_sb = w1p.tile([P, KT, 3, D], bf16)
        wip = tc.alloc_tile_pool(name="wip", bufs=1)
        wi_sb = wip.tile([P, KT, D], bf16)
        xp = tc.alloc_tile_pool(name="xp", bufs=2)
        # right stack: S below hln (hln released at A end, S after expand)
        S_pool = tc.alloc_tile_pool(name="S", bufs=1, side="right")
        S_sb = S_pool.tile([P, LT, D], f16)
        hln_pool = tc.alloc_tile_pool(name="hln", bufs=1, side="right")
        hln_sb = hln_pool.tile([P, LT, D], bf16)

        # zero-init scan boundary slots
        nc.vector.memset(S_sb[0:64, 0, :], 0.0)
        nc.vector.memset(S_sb[64:P, LT - 1, :], 0.0)

        # ---- phase A inputs ----
        xT_r = xT.rearrange("(kt p) l -> kt p l", p=P)
        wi_r = Wi.rearrange("(kt p) d -> kt p d", p=P)

        def load_xblock(b):
            t = xp.tile([P, KT, XW], bf16, tag="xb", name=f"xb{b}")
            for kt in range(KT):
                nc.sync.dma_start(out=t[:, kt, :],
                                  in_=xT_r[kt][:, b * XW:(b + 1) * XW])
            return t

        xtiles = [load_xblock(0)]
        # eh0 halves first: the first proj group only needs these + block 0
        for kt in range(KT):
            nc.sync.dma_start(out=wi_sb[:, kt, 0:ND], in_=wi_r[kt][:, 0:ND])
        xtiles.append(load_xblock(1))
        for kt in range(KT):
            nc.sync.dma_start(out=wi_sb[:, kt, ND:D], in_=wi_r[kt][:, ND:D])

        # ---- phase A (+ states + intra, pipelined one chunk behind) ----
        def states_intra(lt):
            # SSM chunk states: S = Vp^T @ h_chunk
            for sh in range(EH):
                ps = psI.tile([P, ND], f32, tag="pi", name="pi")
                nc.tensor.matmul(ps[:], lhsT=Vp_sb[:],
                                 rhs=hln_sb[:, lt, sh * ND:(sh + 1) * ND],
                                 start=True, stop=True)
                sl = sh * ND
                if lt < LT - 1:
                    nc.scalar.activation(
                        out=S_sb[0:64, lt + 1, sl:sl + ND], in_=ps[0:64, :],
                        func=AF.Copy)
                if lt >= 1:
                    nc.scalar.activation(
                        out=S_sb[64:P, lt - 1, sl:sl + ND], in_=ps[64:P, :],
                        func=AF.Copy)
            # intra-chunk Toeplitz, ns folded at eviction
            for dh in range(2):
                ps = psI.tile([P, ND], f32, tag="pi", name="pi")
                for k in range(4):
                    dt = dh * 4 + k
                    nc.tensor.matmul(ps[:, k * P:(k + 1) * P],
                                     lhsT=hln_sb[:, lt, dt * P:(dt + 1) * P],
                                     rhs=Tl_sb[:], start=True, stop=True)
                for k in range(4):
                    dt = dh * 4 + k
                    nc.scalar.activation(
                        out=h2T_sb[:, dt, lt * P:(lt + 1) * P],
                        in_=ps[:, k * P:(k + 1) * P],
                        func=AF.Identity, scale=ns_sb[:, dt:dt + 1])

        for lt in range(LT):
            xb = lt // (LT // XB)
            xo = (lt % (LT // XB)) * P
            xblk = xtiles[xb]
            if xo == 0 and xb >= 1 and xb + 1 < XB:
                xtiles.append(load_xblock(xb + 1))
            ps_a = []
            for eh in range(EH):
                ps = psA.tile([P, ND], f32, tag="ps", name="ps")
                for kt in range(KT):
                    nc.tensor.matmul(ps[:],
                                     lhsT=xblk[:, kt, xo:xo + P],
                                     rhs=wi_sb[:, kt, eh * ND:(eh + 1) * ND],
                                     start=(kt == 0),
                                     stop=(kt == KT - 1 and not use_bi))
                if use_bi:
                    nc.tensor.matmul(ps[:], lhsT=ones_sb[:],
                                     rhs=bi_bf[:, eh * ND:(eh + 1) * ND],
                                     start=False, stop=True)
                ps_a.append(ps)
            # LN1 stats straight from PSUM
            stats = statp.tile([P, EH, 6], f32, tag="st", name="st")
            for eh in range(EH):
                nc.vector.bn_stats(out=stats[:, eh, :], in_=ps_a[eh][:])
            mv = statp.tile([P, 2], f32, tag="mv", name="mv")
            nc.vector.bn_aggr(out=mv[:], in_=stats[:])
            rstd = statp.tile([P, 1], f32, tag="rs", name="rs")
            nc.scalar.activation(out=rstd[:], in_=mv[:, 1:2], func=AF.Sqrt,
                                 bias=eps_sb[:], scale=1.0)
            nc.vector.reciprocal(out=rstd[:], in_=rstd[:])
            for eh in range(EH):
                nc.vector.tensor_scalar(
                    out=hln_sb[:, lt, eh * ND:(eh + 1) * ND],
                    in0=ps_a[eh][:], scalar1=mv[:, 0:1], scalar2=rstd[:],
                    op0=OP.subtract, op1=OP.mult)
            # SBUF->SBUF transposed copy of this chunk
            nc.scalar.dma_start_transpose(
                out=hlnT_sb[:, :, lt * P:(lt + 1) * P], in_=hln_sb[:, lt, :])
            # SSM states + intra for the PREVIOUS chunk: gives the DVE LN
            # chain one proj-tile of slack so the PE never waits on it.
            if lt >= 1:
                states_intra(lt - 1)
            # stream w1 tiles 0..5 during A (gpsimd queue: keep the sync
            # queue free for the xT block prefetches)
            if lt in (4, 6, 8, 10, 12, 14):
                it = (lt - 4) // 2
                nc.gpsimd.dma_start(out=w1_sb[:, it, :, :], in_=w1T[it])

        states_intra(LT - 1)
        xp.release()
        hln_pool.release()
        for it in (6, 7):
            nc.gpsimd.dma_start(out=w1_sb[:, it, :, :], in_=w1T[it])

        # ---- chunk scans (DVE, fp16, in-place) ----
        for t in range(1, LT):
            nc.vector.scalar_tensor_tensor(
                out=S_sb[0:64, t, :], in0=S_sb[0:64, t - 1, :],
                scalar=dk_sb[0:64, 0:1], in1=S_sb[0:64, t, :],
                op0=OP.mult, op1=OP.add)
            s = LT - 1 - t
            nc.vector.scalar_tensor_tensor(
                out=S_sb[64:P, s, :], in0=S_sb[64:P, s + 1, :],
                scalar=dk_sb[64:P, 0:1], in1=S_sb[64:P, s, :],
                op0=OP.mult, op1=OP.add)

        # ---- conv1 ----
        cop = tc.alloc_tile_pool(name="co", bufs=1)
        co_sb = cop.tile([P, KT, L], bf16)

        def conv_mms(ps, w_sb, src_sb, ot, lc):
            first = True
            for it in range(KT):
                for j in (1, 0, 2):
                    o0 = 1 if (j == 0 and lc == 0) else 0
                    o1 = NF - 1 if (j == 2 and lc == LC - 1) else NF
                    base = lc * NF + j - 1
                    nc.tensor.matmul(
                        ps[:, o0:o1],
                        lhsT=w_sb[:, it, j, ot * P:(ot + 1) * P],
                        rhs=src_sb[:, it, base + o0:base + o1],
                        start=first,
                        stop=(it == KT - 1 and j == 2))
                    first = False

        def conv1_lc(lc):
            for ot in range(KT):
                ps = psA.tile([P, NF], f32, tag="ps", name="ps")
                conv_mms(ps, w1_sb, hlnT_sb, ot, lc)
                nc.scalar.activation(
                    out=co_sb[:, ot, lc * NF:(lc + 1) * NF],
                    in_=ps[:], func=AF.Silu, bias=bc1_sb[:, ot:ot + 1],
                    scale=1.0)

        conv1_lc(0)
        # Wo reuses wi storage (free since A); load during conv1 so it is
        # ready well before the first F tile.
        wo_sb = wi_sb
        wo_r = Wo.rearrange("(dt p) e -> dt p e", p=P)
        for dt in range(KT):
            nc.sync.dma_start(out=wo_sb[:, dt, :], in_=wo_r[dt])
        conv1_lc(1)

        # ---- inter-chunk expand ----
        for i in range(LT):
            for dh in range(2):
                ps = psI.tile([P, ND], f32, tag="pi", name="pi")
                for k in range(4):
                    dt = dh * 4 + k
                    nc.tensor.matmul(ps[:, k * P:(k + 1) * P],
                                     lhsT=S_sb[:, i, dt * P:(dt + 1) * P],
                                     rhs=U_sb[:], start=True, stop=True)
                for k in range(4):
                    dt = dh * 4 + k
                    nc.vector.scalar_tensor_tensor(
                        out=h2T_sb[:, dt, i * P:(i + 1) * P],
                        in0=ps[:, k * P:(k + 1) * P],
                        scalar=ns_sb[:, dt:dt + 1],
                        in1=h2T_sb[:, dt, i * P:(i + 1) * P],
                        op0=OP.mult, op1=OP.add)
        if use_b1mix:
            for dt in range(KT):
                nc.vector.scalar_tensor_tensor(
                    out=h2T_sb[:, dt, :], in0=rs_rep[:],
                    scalar=nb1_sb[:, dt:dt + 1], in1=h2T_sb[:, dt, :],
                    op0=OP.mult, op1=OP.add)
        S_pool.release()

        conv1_lc(2)
        conv1_lc(3)

        # ---- conv2 weights (reuse w1 storage) ----
        w2_sb = w1_sb
        for it in range(KT):
            nc.sync.dma_start(out=w2_sb[:, it, :, :], in_=w2T[it])

        # ---- conv2 + proj-out/LN2 interleaved ----
        hbufp = tc.alloc_tile_pool(name="hbuf", bufs=2)
        x_r = x_res.rearrange("(t p) d -> t p d", p=P)
        out_r = out.rearrange("(t p) d -> t p d", p=P)

        def f_tile(lt):
            x_t = hbufp.tile([P, D], f32, tag="x_t", name="x_t", bufs=2)
            nc.gpsimd.dma_start(out=x_t[:], in_=x_r[lt])
            ps_f = []
            for eh in range(EH):
                ps = psA.tile([P, ND], f32, tag="ps", name="ps")
                for dt in range(KT):
                    nc.tensor.matmul(ps[:],
                                     lhsT=h2T_sb[:, dt, lt * P:(lt + 1) * P],
                                     rhs=wo_sb[:, dt, eh * ND:(eh + 1) * ND],
                                     start=(dt == 0),
                                     stop=(dt == KT - 1 and not use_bo))
                if use_bo:
                    nc.tensor.matmul(ps[:], lhsT=ones_sb[:],
                                     rhs=bo_bf[:, eh * ND:(eh + 1) * ND],
                                     start=False, stop=True)
                ps_f.append(ps)
            stats = statp.tile([P, EH, 6], f32, tag="st", name="st")
            for eh in range(EH):
                nc.vector.bn_stats(out=stats[:, eh, :], in_=ps_f[eh][:])
            mv = statp.tile([P, 2], f32, tag="mv", name="mv")
            nc.vector.bn_aggr(out=mv[:], in_=stats[:])
            rstd = statp.tile([P, 1], f32, tag="rs", name="rs")
            nc.scalar.activation(out=rstd[:], in_=mv[:, 1:2], func=AF.Sqrt,
                                 bias=eps_sb[:], scale=1.0)
            nc.vector.reciprocal(out=rstd[:], in_=rstd[:])
            y = hbufp.tile([P, D], f32, tag="y", name="y", bufs=2)
            for eh in range(EH):
                nc.vector.tensor_scalar(
                    out=y[:, eh * ND:(eh + 1) * ND], in0=ps_f[eh][:],
                    scalar1=mv[:, 0:1], scalar2=rstd[:],
                    op0=OP.subtract, op1=OP.mult)
            if use_g2:
                nc.vector.tensor_mul(out=y[:], in0=y[:], in1=g2_rep[:])
            if use_b2:
                nc.vector.tensor_add(out=y[:], in0=y[:], in1=b2_rep[:])
            out_t = hbufp.tile([P, D], f32, tag="o_t", name="o_t", bufs=2)
            nc.vector.tensor_add(out=out_t[:], in0=y[:], in1=x_t[:])
            nc.scalar.dma_start(out=out_r[lt], in_=out_t[:])

        # F tiles of block lc run interleaved with conv2 of block lc+1 so
        # the LN2 chain and PSUM pressure hide under conv matmuls.
        prev_lc = None
        for lc in range(LC):
            for ot in range(KT):
                ps = psA.tile([P, NF], f32, tag="ps", name="ps")
                conv_mms(ps, w2_sb, co_sb, ot, lc)
                nc.vector.scalar_tensor_tensor(
                    out=h2T_sb[:, ot, lc * NF:(lc + 1) * NF],
                    in0=ps[:], scalar=bc2_sb[:, ot:ot + 1],
                    in1=h2T_sb[:, ot, lc * NF:(lc + 1) * NF],
                    op0=OP.add, op1=OP.add)
                if prev_lc is not None and ot % 2 == 1:
                    f_tile(prev_lc * 4 + ot // 2)
            prev_lc = lc
        for lt in range(prev_lc * 4, prev_lc * 4 + 4):
            f_tile(lt)

        hbufp.release()
        cop.release()
        wip.release()
        w1p.release()
        hlnT_pool.release()
        h2T_pool.release()
        statp.release()
        psI.release()
        psA.release()
        const.release()

    nc.compile()
    return nc


def _bf(a):
    return np.ascontiguousarray(np.asarray(a, np.float32)).astype(_BF16)


def _prep_maps(inputs, L, D, n_cores):
    P = 128
    Q = P
    KT = D // P
    f32 = np.float32
    f64 = np.float64
    x = np.asarray(inputs["x"], f32)
    t = np.asarray(inputs["t"], f32)
    beta1 = float(np.asarray(inputs["beta1"], f32)[0])
    beta2 = float(np.asarray(inputs["beta2"], f32)[0])
    g1 = np.asarray(inputs["g1"], f64)
    b1 = np.asarray(inputs["b1"], f64)

    # SSM params
    af = np.diagonal(np.asarray(inputs["Af"], f64))
    ab = np.diagonal(np.asarray(inputs["Ab"], f64))
    wf = np.asarray(inputs["Bf"], f64)[:, 0] * np.asarray(inputs["Cf"], f64)[0]
    wb = np.asarray(inputs["Bb"], f64)[:, 0] * np.asarray(inputs["Cb"], f64)[0]
    Df = f64(np.asarray(inputs["Df"])[0])
    Db = f64(np.asarray(inputs["Db"])[0])

    lar = np.arange(L, dtype=f64)
    kf = np.exp(lar[:, None] * af[None, :]) @ wf + Df
    kb = np.exp(lar[:, None] * ab[None, :]) @ wb + Db

    # intra-chunk Toeplitz (exact kernels, betas folded), transposed
    p = np.arange(Q)
    dd = p[:, None] - p[None, :]
    Tloc = (np.where(dd >= 0, beta1 * kf[np.clip(dd, 0, None)], 0.0)
            + np.where(dd <= 0, beta2 * kb[np.clip(-dd, 0, None)], 0.0))
    TlT = np.ascontiguousarray(Tloc.T.astype(f32)).astype(_BF16)

    # fold one mode per direction into the constant term
    def fold(a, w, Dv):
        j = np.arange(1, L, dtype=f64)
        errs = [np.abs(w[n] * (np.exp(a[n] * j) - 1.0)).max()
                for n in range(len(a))]
        n0 = int(np.argmin(errs))
        keep = [n for n in range(len(a)) if n != n0]
        return a[keep], w[keep], Dv + w[n0]

    af2, wf2, Df2 = fold(af, wf, Df)
    ab2, wb2, Db2 = fold(ab, wb, Db)
    lf = np.exp(af2)
    lb = np.exp(ab2)

    q = np.arange(Q, dtype=f64)
    V = np.zeros((Q, 128), f64)
    V[:, 0:63] = lf[None, :] ** (Q - q[:, None])
    V[:, 63] = 1.0
    V[:, 64:127] = lb[None, :] ** q[:, None]
    V[:, 127] = 1.0
    V /= _SV
    U = np.zeros((128, Q), f64)
    U[0:63] = (beta1 * wf2[:, None]) * lf[:, None] ** p[None, :]
    U[63] = beta1 * Df2
    U[64:127] = (beta2 * wb2[:, None]) * lb[:, None] ** (Q - p[None, :])
    U[127] = beta2 * Db2
    U *= _SV
    dk = np.concatenate([lf ** Q, [1.0], lb ** Q, [1.0]]).astype(f32)

    # timestep embedding -> noise scale, g1 folded in
    half = D // 2
    freqs = np.exp(np.arange(half, dtype=f32)
                   * (-math.log(10000.0) / (half - 1)))
    ang = t[:, None] * freqs[None, :]
    emb = np.concatenate([np.sin(ang), np.cos(ang)], axis=1).astype(f32)
    ns = (1.0 / (1.0 + np.exp(-emb))).astype(f64) * g1[None, :]  # (B, D)

    # conv weights: g1 folded into w1 input channels, b1 into bc1
    w1 = np.asarray(inputs["w1"], f64)
    w2 = np.asarray(inputs["w2"], f64)
    w1f = w1 * g1[None, :, None]
    bc1 = np.asarray(inputs["bc1"], f64) + w1.sum(axis=2) @ b1
    bc2 = np.asarray(inputs["bc2"], f64)
    w1T = np.ascontiguousarray(np.transpose(w1f.astype(f32), (1, 2, 0))
                               ).reshape(KT, P, 3, D).astype(_BF16)
    w2T = np.ascontiguousarray(np.transpose(w2.astype(f32), (1, 2, 0))
                               ).reshape(KT, P, 3, D).astype(_BF16)

    use_bi = bool(np.any(np.asarray(inputs["bi"], f32) != 0))
    use_bo = bool(np.any(np.asarray(inputs["bo"], f32) != 0))
    use_g2 = bool(np.any(np.asarray(inputs["g2"], f32) != 1))
    use_b2 = bool(np.any(np.asarray(inputs["b2"], f32) != 0))
    use_b1mix = bool(np.any(b1 != 0))
    flags = (use_bi, use_bo, use_g2, use_b2, use_b1mix)

    # b1-through-mix rank-1 term (only if b1 != 0)
    tms = np.arange(L)[:, None] - np.arange(L)[None, :]
    if use_b1mix:
        Tf_rs = np.where(tms >= 0, kf[np.clip(tms, 0, None)], 0.0).sum(1)
        Tb_rs = np.where(tms <= 0, kb[np.clip(-tms, 0, None)], 0.0).sum(1)
        rowsum = (beta1 * Tf_rs + beta2 * Tb_rs).astype(f32)
    else:
        rowsum = np.zeros(L, f32)

    def col(v):
        return np.ascontiguousarray(np.asarray(v, f32).reshape(KT, P).T)

    shared = {
        "Wi": _bf(inputs["Wi"]), "Wo": _bf(inputs["Wo"]),
        "w1T": w1T, "w2T": w2T,
        "Vp": np.ascontiguousarray(V.astype(f32)).astype(_BF16),
        "Up": np.ascontiguousarray(U.astype(f32)).astype(_F16),
        "TlT": TlT,
        "dkc": np.ascontiguousarray(dk.reshape(P, 1)),
        "bc1c": col(bc1.astype(f32)), "bc2c": col(bc2.astype(f32)),
    }
    if use_bi:
        shared["biR"] = np.asarray(inputs["bi"], f32).reshape(1, D)
    if use_bo:
        shared["boR"] = np.asarray(inputs["bo"], f32).reshape(1, D)
    if use_g2:
        shared["g2R"] = np.asarray(inputs["g2"], f32).reshape(1, D)
    if use_b2:
        shared["b2R"] = np.asarray(inputs["b2"], f32).reshape(1, D)
    if use_b1mix:
        shared["rsR"] = rowsum.reshape(1, L)
    in_maps = []
    for b in range(n_cores):
        xb = np.ascontiguousarray(x[b])
        m = dict(shared)
        m["x_res"] = xb
        m["xT"] = np.ascontiguousarray(xb.T.astype(_BF16))
        m["nsc"] = col(ns[b].astype(f32))
        if use_b1mix:
            m["nb1c"] = col((ns[b] * b1).astype(f32))
        in_maps.append(m)
    return in_maps, flags


def get_nc(L=_L, D=_D, n_cores=_B, flags=(False,) * 5):
    key = (L, D, n_cores, flags)
    if key not in _cache:
        _cache[key] = _build(L, D, n_cores, flags)
    return _cache[key]


def kernel(**inputs):
    from concourse.bass_utils import run_bass_kernel_spmd

    L, D, B = _L, _D, _B
    in_maps, flags = _prep_maps(inputs, L, D, B)
    nc = get_nc(L, D, B, flags)
    res = run_bass_kernel_spmd(nc, in_maps, core_ids=list(range(B)))
    return np.stack([res.results[c]["out"] for c in range(B)]).astype(
        np.float32)
